# revision 2
# baseline (speedup 1.0000x reference)
"""Trainium2 Bass kernel for nn_Cp_linear_2D (CP/low-rank linear layer).

reference: W = einsum("ir,r,or->io", U1, lam, U2); y = x @ W + bias
  x: [4096, 4096], U1: [4096, 256], U2: [4096, 256], lam: [256], bias: [4096]

Strategy (8 cores, data-parallel over batch):
  - Never materialize W. Factored form: y = ((x @ (U1*lam)) @ U2.T + bias
    (17 GFLOP instead of 154 GFLOP). lam is folded into U1 on the host.
  - Each core gets a 512-row batch shard of x, pre-transposed on host to
    xT [4096, 512] so the contraction dim (IN) lands on SBUF partitions.
  - Stage A: z[r, b] = sum_k U1L[k, r] * xT[k, b] -> z [256, 512] fp16.
  - Stage B: y'[n, b] = sum_r U2T[r, n] * z[r, b] + bias[n]. Output is
    produced n-major ([OUT, BSH] per core) so the bias is a per-partition
    scalar: PSUM eviction alternates between DVE (tensor_tensor add) and
    ACT (activation Identity with per-partition bias AP), keeping both
    engines off the critical path. The host transposes each core's
    [4096, 512] result back to [512, 4096] when assembling the output.

MODE selects numerics (default fp16):
  - "fp16": all streams fp16 (x, U1*lam, U2T, z, y'), fp32 PSUM
    accumulate. ~2.6e-4 rms / ~1.5e-3 max rel error vs fp64 - far inside
    the 2e-2 gate. Halves DMA traffic vs fp32/fp32r (12.6 MB/core vs 24).
  - "fp32rc": compensated fp32r - operands Dekker-split into f32r hi+lo;
    3-pass matmuls. fp32-class error (~3e-7). ~105us. Fallback mode.
  - "fp32":   native fp32 matmuls (4 cyc/row). ~121us, ~2.4e-7. Last
    fallback for the retry wrapper.
  - "fp32r":  tf32-like single pass, host-rounded. ~82us, ~2.2e-4.
  - "bf16x3": bf16 hi+lo host-split, 3 passes. ~99us, ~7e-6.
"""

from contextlib import ExitStack

import numpy as np

import concourse.bass as bass
import concourse.mybir as mybir
import concourse.tile as tile
from concourse import bacc

P = 128
B, IN, OUT, R = 4096, 4096, 4096, 256
NCORES = 8
BSH = B // NCORES          # 512 batch rows per core
KT = IN // P               # 32 k-tiles for stage A
RT = R // P                # 2 r-tiles
MB = BSH // P              # 4 output row tiles per core (legacy modes)
NT = OUT // 512            # 8 output column tiles of 512 (legacy modes)
NTP = OUT // P             # 32 output partition-tiles (fp16 mode)

F32 = mybir.dt.float32
F32R = mybir.dt.float32r
F16 = mybir.dt.float16
BF16 = mybir.dt.bfloat16

MODE = "fp16"


# --------------------------------------------------------------------------
# fp16 kernel (primary)
# --------------------------------------------------------------------------

def build_kernel_fp16():
    """Single-pass fp16 kernel, output in [OUT, BSH] (n-major) layout."""
    nc = bacc.Bacc(
        "TRN2", target_bir_lowering=False, debug=False, enable_asserts=False
    )
    xT = nc.dram_tensor("xT", [IN, BSH], F16, kind="ExternalInput").ap()
    U1L = nc.dram_tensor("U1L", [IN, R], F16, kind="ExternalInput").ap()
    U2T = nc.dram_tensor("U2T", [R, OUT], F16, kind="ExternalInput").ap()
    biasP = nc.dram_tensor("biasP", [P, NTP], F32, kind="ExternalInput").ap()
    y = nc.dram_tensor("y", [OUT, BSH], F16, kind="ExternalOutput").ap()

    U2T_r = U2T.rearrange("(t p) n -> p t n", p=P)

    with tile.TileContext(nc) as tc:
        with ExitStack() as ctx:
            const = ctx.enter_context(tc.tile_pool(name="const", bufs=1))
            xpool = ctx.enter_context(tc.tile_pool(name="xpool", bufs=3))
            u1pool = ctx.enter_context(tc.tile_pool(name="u1pool", bufs=3))
            ypool = ctx.enter_context(tc.tile_pool(name="ypool", bufs=3))
            psumA = ctx.enter_context(
                tc.tile_pool(name="psumA", bufs=1, space="PSUM"))
            psumB = ctx.enter_context(
                tc.tile_pool(name="psumB", bufs=6, space="PSUM"))

            CK = 4                      # k-tiles per x/U1 DMA chunk
            NCH = KT // CK              # 8 chunks
            UCS = 512                   # U2T chunk width (n columns)

            psA = [psumA.tile([P, BSH], F32, name=f"psA{m}") for m in range(RT)]
            bias_sb = const.tile([P, NTP], F32)
            u2_sb = const.tile([P, RT, OUT], F16)
            z_sb = const.tile([P, RT, BSH], F16)

            # --- stage A: z[r, b] = sum_k U1L[k, r] xT[k, b]
            for ck in range(NCH):
                k0 = ck * CK
                xt = xpool.tile([P, CK, BSH], F16, tag="xt", name=f"xt_{ck}")
                nc.sync.dma_start(
                    xt[:],
                    xT[k0 * P:(k0 + CK) * P, :].rearrange("(t p) b -> p t b", p=P),
                )
                u1t = u1pool.tile([P, CK, R], F16, tag="u1t", name=f"u1t_{ck}")
                nc.sync.dma_start(
                    u1t[:],
                    U1L[k0 * P:(k0 + CK) * P, :].rearrange("(t p) r -> p t r", p=P),
                )
                if ck == 0:
                    nc.sync.dma_start(bias_sb[:], biasP)
                for t in range(CK):
                    k = k0 + t
                    for m in range(RT):
                        nc.tensor.matmul(
                            psA[m][:],
                            u1t[:, t, m * P:(m + 1) * P],
                            xt[:, t, :],
                            start=(k == 0),
                            stop=(k == KT - 1),
                        )

            # U2T loads issue after all x/U1 loads; FIFO transfer order means
            # chunk ci arrives well before stage B consumes it.
            for ci in range(OUT // UCS):
                sl = slice(ci * UCS, (ci + 1) * UCS)
                nc.sync.dma_start(u2_sb[:, :, sl], U2T_r[:, :, sl])

            # --- z eviction: PSUM fp32 -> SBUF fp16, split DVE/ACT
            nc.vector.tensor_copy(z_sb[:, 0], psA[0][:])
            nc.scalar.copy(z_sb[:, 1], psA[1][:])

            # --- stage B: y'[n, b] = sum_r U2T[r, n] z[r, b] + bias[n]
            YG = 4  # n-tiles per output store
            for nt in range(NTP):
                ps = psumB.tile([P, BSH], F32, tag="psB")
                for kr in range(RT):
                    nc.tensor.matmul(
                        ps[:],
                        u2_sb[:, kr, nt * P:(nt + 1) * P],
                        z_sb[:, kr],
                        start=(kr == 0),
                        stop=(kr == RT - 1),
                    )
                j = nt % YG
                if j == 0:
                    y_sb = ypool.tile([P, YG, BSH], F16, tag="y", name=f"y_{nt}")
                if nt % 2 == 0:
                    nc.vector.tensor_tensor(
                        y_sb[:, j], ps[:],
                        bias_sb[:, nt:nt + 1].to_broadcast((P, BSH)),
                        mybir.AluOpType.add,
                    )
                else:
                    nc.scalar.activation(
                        y_sb[:, j], ps[:],
                        mybir.ActivationFunctionType.Identity,
                        bias=bias_sb[:, nt:nt + 1],
                    )
                if j == YG - 1:
                    g0 = (nt + 1 - YG) * P
                    nc.gpsimd.dma_start(
                        y[g0:(nt + 1) * P, :].rearrange("(t p) b -> p t b", p=P),
                        y_sb[:],
                    )

    nc.compile()
    return nc


# --------------------------------------------------------------------------
# legacy fp32-class kernels (fallback modes)
# --------------------------------------------------------------------------

def _stage_a_matmuls(nc, ps, lhs_tiles, rhs_tiles, k, last_k):
    """Accumulate all operand-split combinations for one k tile into ps."""
    combos = []
    if len(lhs_tiles) == 1:
        combos = [(0, 0)]
    else:  # hi*hi, hi*lo, lo*hi
        combos = [(0, 0), (0, 1), (1, 0)]
    for ci, (li, ri) in enumerate(combos):
        nc.tensor.matmul(
            ps, lhs_tiles[li], rhs_tiles[ri],
            start=(k == 0 and ci == 0),
            stop=(k == last_k and ci == len(combos) - 1),
        )


def build_kernel(mode=None):
    mode = mode or MODE
    if mode == "fp16":
        return build_kernel_fp16()
    nc = bacc.Bacc(
        "TRN2", target_bir_lowering=False, debug=False, enable_asserts=False
    )
    mm_dt = {"fp32": F32, "fp32r": F32R, "bf16x3": BF16, "fp32rc": F32R}[mode]
    nsplit = 2 if mode in ("bf16x3", "fp32rc") else 1
    # fp32rc: all inputs arrive as single fp32 streams (no DMA inflation);
    # hi/lo Dekker splits into f32r happen on-device, with the copy/sub
    # passes balanced across POOL and DVE.
    x_dev_split = mode == "fp32rc"
    x_streams = 2 if (nsplit == 2 and not x_dev_split) else 1
    w_streams = 1 if x_dev_split else nsplit
    x_dt = F32 if x_dev_split else mm_dt
    w_dt = F32 if x_dev_split else mm_dt

    if x_streams == 1:
        xTs = [nc.dram_tensor("xT", [IN, BSH], x_dt, kind="ExternalInput").ap()]
    else:
        xTs = [nc.dram_tensor(f"xT_{sfx}", [IN, BSH], mm_dt,
                              kind="ExternalInput").ap() for sfx in ("hi", "lo")]
    if w_streams == 1:
        U1s = [nc.dram_tensor("U1", [IN, R], w_dt, kind="ExternalInput").ap()]
        U2Ts = [nc.dram_tensor("U2T", [R, OUT], w_dt, kind="ExternalInput").ap()]
    else:
        U1s = [nc.dram_tensor(f"U1_{sfx}", [IN, R], mm_dt,
                              kind="ExternalInput").ap() for sfx in ("hi", "lo")]
        U2Ts = [nc.dram_tensor(f"U2T_{sfx}", [R, OUT], mm_dt,
                               kind="ExternalInput").ap() for sfx in ("hi", "lo")]
    lam = nc.dram_tensor("lam", [R], F32, kind="ExternalInput").ap()
    bias = nc.dram_tensor("bias", [OUT], F32, kind="ExternalInput").ap()
    y = nc.dram_tensor("y", [BSH, OUT], F32, kind="ExternalOutput").ap()

    with tile.TileContext(nc) as tc:
        with ExitStack() as ctx:
            const = ctx.enter_context(tc.tile_pool(name="const", bufs=1))
            raw_bufs = 2 if x_dev_split else 6
            xpool = ctx.enter_context(tc.tile_pool(name="xpool", bufs=raw_bufs))
            wpool = ctx.enter_context(tc.tile_pool(name="wpool", bufs=raw_bufs))
            spool = ctx.enter_context(
                tc.tile_pool(name="spool", bufs=3))
            u2rpool = ctx.enter_context(tc.tile_pool(name="u2rpool", bufs=2))
            zpool = ctx.enter_context(tc.tile_pool(name="zpool", bufs=1))
            ypool = ctx.enter_context(
                tc.tile_pool(name="ypool", bufs=3 if x_dev_split else 4))
            psumA = ctx.enter_context(tc.tile_pool(name="psumA", bufs=1, space="PSUM"))
            psumB = ctx.enter_context(tc.tile_pool(name="psumB", bufs=4, space="PSUM"))

            # --- stage A: z[r, b] = sum_k U1[k, r] xT[k, b]  (K = IN = 4096)
            # DMAs are batched into multi-k-tile chunks: descriptor-gen cost
            # (~625ns/DMA on the shared HWDGE) is per dma_start, so fewer +
            # bigger transfers keep the DMA pipeline at bandwidth. The first
            # chunk is small so PE starts early. Constant loads (u2, bias,
            # lam) are sprinkled in so they fill otherwise-idle DMA time.
            CHUNKS = [1, 3] + [4] * 7  # k-tiles per DMA chunk, sum = KT
            CMAX = max(CHUNKS)
            psA = [psumA.tile([P, BSH], F32, name=f"psA{m}") for m in range(RT)]
            lam_sb = const.tile([P, RT], F32)
            bias_bc = const.tile([P, OUT], F32)
            u2_sb = [const.tile([P, RT, OUT], mm_dt, tag=f"u2{s}", name=f"u2{s}")
                     for s in range(nsplit)]
            U2T_r = [u.rearrange("(kt p) n -> p kt n", p=P) for u in U2Ts]
            # u2 load chunk width. For fp32rc most u2 split work is deferred
            # into stage B (where DVE is otherwise idle), chunk = one n-tile.
            csz = 512 if x_dev_split else 1024
            u2_jobs = [(s, ci) for s in range(w_streams)
                       for ci in range(OUT // csz)]
            n_u2_stage_a = 3 if x_dev_split else len(u2_jobs)

            def load_u2_chunk(s, ci):
                sl = slice(ci * csz, (ci + 1) * csz)
                if not x_dev_split:
                    nc.sync.dma_start(u2_sb[s][:, :, sl], U2T_r[s][:, :, sl])
                    return
                raw = u2rpool.tile([P, RT, csz], F32, tag="u2raw",
                                   name=f"u2raw_{ci}")
                nc.sync.dma_start(raw[:], U2T_r[0][:, :, sl])
                nc.vector.tensor_copy(u2_sb[0][:, :, sl], raw[:])
                nc.vector.tensor_tensor(
                    u2_sb[1][:, :, sl], raw[:], u2_sb[0][:, :, sl],
                    mybir.AluOpType.subtract,
                )

            k0 = 0
            for ck, clen in enumerate(CHUNKS):
                xts, u1ts = [], []
                for s in range(x_streams):
                    xt = xpool.tile([P, CMAX, BSH], x_dt, tag=f"xt{s}",
                                    name=f"xt{s}_{ck}")
                    nc.sync.dma_start(
                        xt[:, :clen],
                        xTs[s][k0 * P:(k0 + clen) * P, :]
                        .rearrange("(t p) b -> p t b", p=P),
                    )
                    xts.append(xt)
                for s in range(w_streams):
                    u1t = wpool.tile([P, CMAX, R], w_dt, tag=f"u1{s}",
                                     name=f"u1{s}_{ck}")
                    nc.sync.dma_start(
                        u1t[:, :clen],
                        U1s[s][k0 * P:(k0 + clen) * P, :]
                        .rearrange("(t p) r -> p t r", p=P),
                    )
                    u1ts.append(u1t)
                if x_dev_split:
                    # Dekker split on device (DVE): hi = rnd_f32r(x),
                    # lo = rnd_f32r(x - hi); chunk-granular ops.
                    xh = spool.tile([P, CMAX, BSH], F32R, tag="xh",
                                    name=f"xh_{ck}")
                    xl = spool.tile([P, CMAX, BSH], F32R, tag="xl",
                                    name=f"xl_{ck}")
                    u1h = u2rpool.tile([P, CMAX, R], F32R, tag="u1h",
                                       name=f"u1h_{ck}")
                    u1l = u2rpool.tile([P, CMAX, R], F32R, tag="u1l",
                                       name=f"u1l_{ck}")
                    nc.vector.tensor_copy(xh[:, :clen], xts[0][:, :clen])
                    nc.vector.tensor_tensor(
                        xl[:, :clen], xts[0][:, :clen], xh[:, :clen],
                        mybir.AluOpType.subtract,
                    )
                    nc.vector.tensor_copy(u1h[:, :clen], u1ts[0][:, :clen])
                    nc.vector.tensor_tensor(
                        u1l[:, :clen], u1ts[0][:, :clen], u1h[:, :clen],
                        mybir.AluOpType.subtract,
                    )
                    xts = [xh, xl]
                    u1ts = [u1h, u1l]
                if ck == 1:
                    nc.sync.dma_start(lam_sb[:], lam.rearrange("(o p) -> p o", p=P))
                    nc.sync.dma_start(bias_bc[0:1, :],
                                      bias.rearrange("(a n) -> a n", a=1))
                if ck == (len(CHUNKS) - 1 if x_dev_split else 2):
                    nc.gpsimd.partition_broadcast(bias_bc[:], bias_bc[0:1, :])
                if ck >= 3 and u2_jobs and (
                        (OUT // csz) * w_streams - len(u2_jobs) < n_u2_stage_a):
                    load_u2_chunk(*u2_jobs.pop(0))
                    if ck == len(CHUNKS) - 1 and not x_dev_split:
                        while u2_jobs:  # flush any leftovers (bf16x3)
                            load_u2_chunk(*u2_jobs.pop(0))
                for t in range(clen):
                    k = k0 + t
                    for m in range(RT):
                        _stage_a_matmuls(
                            nc, psA[m][:],
                            [u[:, t, m * P:(m + 1) * P] for u in u1ts],
                            [x[:, t, :] for x in xts],
                            k, KT - 1,
                        )
                k0 += clen

            # --- z eviction: z = psA * lam  (+ hi/lo split on device for bf16x3)
            z_sb = []
            if nsplit == 1:
                zt = zpool.tile([P, RT, BSH], mm_dt, tag="z")
                for m in range(RT):
                    nc.vector.tensor_tensor(
                        zt[:, m], psA[m][:],
                        lam_sb[:, m:m + 1].to_broadcast((P, BSH)),
                        mybir.AluOpType.mult,
                    )
                z_sb = [zt]
            else:
                lo_dt = F32R if x_dev_split else BF16
                zf = zpool.tile([P, RT, BSH], F32, tag="zf")
                z_hi = zpool.tile([P, RT, BSH], lo_dt, tag="zhi")
                z_lo = zpool.tile([P, RT, BSH], lo_dt, tag="zlo")
                for m in range(RT):
                    nc.vector.tensor_tensor(
                        zf[:, m], psA[m][:],
                        lam_sb[:, m:m + 1].to_broadcast((P, BSH)),
                        mybir.AluOpType.mult,
                    )
                    nc.vector.tensor_copy(z_hi[:, m], zf[:, m])
                    nc.vector.tensor_tensor(
                        z_lo[:, m], zf[:, m], z_hi[:, m],
                        mybir.AluOpType.subtract,
                    )
                z_sb = [z_hi, z_lo]

            # --- stage B: y[b, n] = sum_r z[r, b] U2T[r, n] + bias[n]
            # y goes out in [P, 2048] chunks via gpsimd (SWDGE) so store
            # descriptor-gen doesn't contend with the HWDGE load pipeline.
            combos = [(0, 0)] if nsplit == 1 else [(0, 0), (0, 1), (1, 0)]
            YC = 1024  # columns per output store
            for mb in range(MB):
                y_sb = None
                for n in range(NT):
                    # fp32rc: remaining u2 chunks split just-in-time here —
                    # chunk n+2 is produced while tiles (mb=0, n) compute,
                    # keeping the big u2 split off stage A's critical DVE path.
                    if u2_jobs:
                        load_u2_chunk(*u2_jobs.pop(0))
                    ps = psumB.tile([P, 512], F32, tag="psB")
                    for kr in range(RT):
                        for ci, (li, ri) in enumerate(combos):
                            nc.tensor.matmul(
                                ps[:],
                                z_sb[li][:, kr, mb * P:(mb + 1) * P],
                                u2_sb[ri][:, kr, n * 512:(n + 1) * 512],
                                start=(kr == 0 and ci == 0),
                                stop=(kr == RT - 1 and ci == len(combos) - 1),
                            )
                    # last row-block streams out per 512-col tile so the
                    # kernel tail is one small store, not a 1MB one
                    yc = 512 if (mb == MB - 1 and mm_dt == F32) else YC
                    if n % (yc // 512) == 0:
                        y_sb = ypool.tile([P, yc], F32, tag="y", name=f"y_{mb}_{n}")
                    off = (n % (yc // 512)) * 512
                    nc.vector.tensor_tensor(
                        y_sb[:, off:off + 512], ps[:],
                        bias_bc[:, n * 512:(n + 1) * 512],
                        mybir.AluOpType.add,
                    )
                    if (n + 1) % (yc // 512) == 0:
                        nc.gpsimd.dma_start(
                            y[mb * P:(mb + 1) * P,
                              (n + 1) * 512 - yc:(n + 1) * 512],
                            y_sb[:, :yc],
                        )

    nc.compile()
    return nc


def _round_f32r(a):
    """Round fp32 -> tf32-like (11 mantissa bits), round-half-to-even."""
    bits = a.view(np.uint32)
    rounded = (bits.astype(np.uint64) + 0x7FF + ((bits >> 12) & 1)) & 0xFFFFF000
    return rounded.astype(np.uint32).view(np.float32)


def _split_bf16(a):
    import ml_dtypes
    hi = a.astype(ml_dtypes.bfloat16)
    lo = (a - hi.astype(np.float32)).astype(ml_dtypes.bfloat16)
    return hi, lo


def prep_in_maps(x, U1, U2, lam, bias, mode=None):
    mode = mode or MODE
    x = np.ascontiguousarray(np.asarray(x, dtype=np.float32))
    U1 = np.ascontiguousarray(np.asarray(U1, dtype=np.float32))
    U2T = np.ascontiguousarray(np.asarray(U2, dtype=np.float32).T)
    lam = np.ascontiguousarray(np.asarray(lam, dtype=np.float32))
    bias = np.ascontiguousarray(np.asarray(bias, dtype=np.float32))

    if mode == "fp16":
        U1L16 = np.ascontiguousarray((U1 * lam[None, :]).astype(np.float16))
        U2T16 = np.ascontiguousarray(U2T.astype(np.float16))
        biasP = np.ascontiguousarray(bias.reshape(NTP, P).T)  # [128, 32] f32
        in_maps = []
        for i in range(NCORES):
            xT16 = np.ascontiguousarray(
                x[i * BSH:(i + 1) * BSH, :].T.astype(np.float16))
            in_maps.append({"xT": xT16, "U1L": U1L16, "U2T": U2T16,
                            "biasP": biasP})
        return in_maps

    in_maps = []
    for i in range(NCORES):
        xT_i = np.ascontiguousarray(x[i * BSH:(i + 1) * BSH, :].T)
        if mode == "fp32":
            m = {"xT": xT_i, "U1": U1, "U2T": U2T}
        elif mode == "fp32rc":
            m = {"xT": xT_i, "U1": U1, "U2T": U2T}
        elif mode == "fp32r":
            m = {"xT": _round_f32r(xT_i), "U1": _round_f32r(U1),
                 "U2T": _round_f32r(U2T)}
        else:
            xh, xl = _split_bf16(xT_i)
            u1h, u1l = _split_bf16(U1)
            u2h, u2l = _split_bf16(U2T)
            m = {"xT_hi": xh, "xT_lo": xl, "U1_hi": u1h, "U1_lo": u1l,
                 "U2T_hi": u2h, "U2T_lo": u2l}
        m["lam"] = lam
        m["bias"] = bias
        in_maps.append(m)
    return in_maps


def postprocess(raw, mode=None):
    """raw: concat over cores along axis 0 of each core's output tensor."""
    mode = mode or MODE
    if mode != "fp16":
        return raw  # already [B, OUT] fp32 in batch order
    out = np.empty((B, OUT), np.float32)
    for i in range(NCORES):
        out[i * BSH:(i + 1) * BSH, :] = raw[i * OUT:(i + 1) * OUT, :].T
    return out


_NC_CACHE = {}


def _build_runner(nc):
    """PJRT runner WITHOUT output-buffer donation.

    The stock run_bass_via_pjrt path donates zero-initialized output
    buffers into the bass_exec custom call; on this axon stack that
    donation intermittently corrupted outputs or crashed the exec unit
    (~1 in 5 fresh-process runs for larger-input kernels). With donation
    off (fresh result buffers, 23/23 clean trials), execution is
    deterministic. Our kernel writes every output element, so the zero
    initial contents are irrelevant.
    """
    import jax
    from jax.sharding import Mesh, PartitionSpec, NamedSharding
    from jax.experimental.shard_map import shard_map
    from concourse import bass2jax

    bass2jax.install_neuronx_cc_hook()
    partition_name = nc.partition_id_tensor.name if nc.partition_id_tensor else None
    in_names, out_names, out_avals, zero_outs = [], [], [], []
    for alloc in nc.m.functions[0].allocations:
        if not isinstance(alloc, mybir.MemoryLocationSet):
            continue
        name = alloc.memorylocations[0].name
        if alloc.kind == "ExternalInput":
            if name != partition_name:
                in_names.append(name)
        elif alloc.kind == "ExternalOutput":
            out_names.append(name)
            shape = tuple(alloc.tensor_shape)
            dtype = mybir.dt.np(alloc.dtype)
            out_avals.append(jax.core.ShapedArray(shape, dtype))
            zero_outs.append(np.zeros(shape, dtype))
    all_in_names = list(in_names) + list(out_names)
    if partition_name is not None:
        all_in_names.append(partition_name)

    def _body(*args):
        operands = list(args)
        if partition_name is not None:
            operands.append(bass2jax.partition_id_tensor())
        return tuple(bass2jax._bass_exec_p.bind(
            *operands,
            out_avals=tuple(out_avals),
            in_names=tuple(all_in_names),
            out_names=tuple(out_names),
            lowering_input_output_aliases=(),
            sim_require_finite=True,
            sim_require_nnan=True,
            nc=nc,
        ))

    devices = jax.devices()[:NCORES]
    mesh = Mesh(np.asarray(devices), ("core",))
    nin = len(in_names) + len(zero_outs)
    fn = jax.jit(
        shard_map(_body, mesh=mesh,
                  in_specs=(PartitionSpec("core"),) * nin,
                  out_specs=(PartitionSpec("core"),) * len(out_names),
                  check_rep=False),
        keep_unused=True,
    )
    sharding = NamedSharding(mesh, PartitionSpec("core"))
    dev_zeros = [
        jax.device_put(
            np.zeros((NCORES * z.shape[0], *z.shape[1:]), z.dtype), sharding)
        for z in zero_outs
    ]

    def run(in_maps):
        concat_in = [
            jax.device_put(
                np.concatenate([np.asarray(in_maps[c][nm])
                                for c in range(NCORES)], axis=0), sharding)
            for nm in in_names
        ]
        outs = fn(*concat_in, *dev_zeros)
        return np.asarray(outs[0])  # concat over cores along axis 0

    return run


_BASS_CACHE = {}


def _run_once(mode, x, U1, U2, lam, bias, _trace, _tmpdir):
    if mode not in _NC_CACHE:
        nc = build_kernel(mode)
        _BASS_CACHE[mode] = nc
        _NC_CACHE[mode] = _build_runner(nc)
    in_maps = prep_in_maps(x, U1, U2, lam, bias, mode)
    return postprocess(_NC_CACHE[mode](in_maps), mode)


def kernel(x, U1, U2, lam, bias, _trace=False, _tmpdir=None, _mode=None):
    # Device execution through the axon tunnel can very occasionally fail
    # transiently (NRT_EXEC_UNIT_UNRECOVERABLE); retry the same mode, then
    # fall back to the legacy fp32-class kernels before giving up.
    mode = _mode or MODE
    attempts = [mode, mode, "fp32rc", "fp32"]
    last_err = None
    for i, m in enumerate(attempts):
        try:
            return _run_once(m, x, U1, U2, lam, bias, _trace, _tmpdir)
        except Exception as e:  # noqa: BLE001 - deliberate retry barrier
            last_err = e
            import time as _time
            _time.sleep(2.0 * (i + 1))
    raise last_err


# revision 30
# speedup vs baseline: 1.1609x; 1.1609x over previous
"""Trainium2 Bass kernel for nn_Cp_linear_2D (CP/low-rank linear layer).

reference: W = einsum("ir,r,or->io", U1, lam, U2); y = x @ W + bias
  x: [4096, 4096], U1: [4096, 256], U2: [4096, 256], lam: [256], bias: [4096]

Strategy (8 cores, data-parallel over batch):
  - Never materialize W. Factored form: y = ((x @ (U1*lam)) @ U2.T + bias
    (17 GFLOP instead of 154 GFLOP). lam is folded into U1 on the host.
  - Each core gets a 512-row batch shard of x, pre-transposed on host to
    xT [4096, 512] so the contraction dim (IN) lands on SBUF partitions.
  - Stage A: z[r, b] = sum_k U1L[k, r] * xT[k, b] -> z [256, 512] fp16.
  - Stage B: y'[n, b] = sum_r U2T[r, n] * z[r, b] + bias[n]. Output is
    produced n-major ([OUT, BSH] per core) so the bias is a per-partition
    scalar: PSUM eviction alternates between DVE (tensor_tensor add) and
    ACT (activation Identity with per-partition bias AP), keeping both
    engines off the critical path. The host transposes each core's
    [4096, 512] result back to [512, 4096] when assembling the output.

MODE selects numerics (default fp16):
  - "fp16": all streams fp16 (x, U1*lam, U2T, z, y'), fp32 PSUM
    accumulate. ~2.6e-4 rms / ~1.5e-3 max rel error vs fp64 - far inside
    the 2e-2 gate. Halves DMA traffic vs fp32/fp32r (12.6 MB/core vs 24).
  - "fp32rc": compensated fp32r - operands Dekker-split into f32r hi+lo;
    3-pass matmuls. fp32-class error (~3e-7). ~105us. Fallback mode.
  - "fp32":   native fp32 matmuls (4 cyc/row). ~121us, ~2.4e-7. Last
    fallback for the retry wrapper.
  - "fp32r":  tf32-like single pass, host-rounded. ~82us, ~2.2e-4.
  - "bf16x3": bf16 hi+lo host-split, 3 passes. ~99us, ~7e-6.
"""

from contextlib import ExitStack

import numpy as np

import concourse.bass as bass
import concourse.mybir as mybir
import concourse.tile as tile
from concourse import bacc

P = 128
B, IN, OUT, R = 4096, 4096, 4096, 256
NCORES = 8
BSH = B // NCORES          # 512 batch rows per core
KT = IN // P               # 32 k-tiles for stage A
RT = R // P                # 2 r-tiles
MB = BSH // P              # 4 output row tiles per core (legacy modes)
NT = OUT // 512            # 8 output column tiles of 512 (legacy modes)
NTP = OUT // P             # 32 output partition-tiles (fp16 mode)

F32 = mybir.dt.float32
F32R = mybir.dt.float32r
F16 = mybir.dt.float16
BF16 = mybir.dt.bfloat16

MODE = "fp16"


# --------------------------------------------------------------------------
# fp16 kernel (primary)
# --------------------------------------------------------------------------

def build_kernel_fp16():
    """Single-pass fp16 kernel, output in [OUT, BSH] (n-major) layout.

    Two-half batch pipeline: the 512-column batch shard is processed as two
    256-column halves so stage B of half 1 runs on PE while half 2 of x
    streams in. DMA order: [x1/U1 interleaved] [U2T] [x2]; half-1 stores go
    out via Pool/SWDGE (independent of SP program order, so they fill DMA
    idle slots during the x2 stream), half-2 stores via SP/HWDGE (lower
    latency, SP is idle by then).
    """
    nc = bacc.Bacc(
        "TRN2", target_bir_lowering=False, debug=False, enable_asserts=False
    )
    xT = nc.dram_tensor("xT", [IN, BSH], F16, kind="ExternalInput").ap()
    U1L = nc.dram_tensor("U1L", [IN, R], F16, kind="ExternalInput").ap()
    U2T = nc.dram_tensor("U2T", [R, OUT], F16, kind="ExternalInput").ap()
    biasP = nc.dram_tensor("biasP", [P, NTP], F32, kind="ExternalInput").ap()
    y = nc.dram_tensor("y", [OUT, BSH], F16, kind="ExternalOutput").ap()

    U2T_r = U2T.rearrange("(t p) n -> p t n", p=P)
    HB = 2
    BH = BSH // HB              # 256 batch columns per half

    with tile.TileContext(nc) as tc:
        with ExitStack() as ctx:
            CHUNKS = [4] * 8
            assert sum(CHUNKS) == KT
            const = ctx.enter_context(tc.tile_pool(name="const", bufs=1))
            xpool = ctx.enter_context(
                tc.tile_pool(name="xpool", bufs=HB * len(CHUNKS)))
            u1pool = ctx.enter_context(
                tc.tile_pool(name="u1pool", bufs=len(CHUNKS)))
            ypool = ctx.enter_context(tc.tile_pool(name="ypool", bufs=8))
            psumA = ctx.enter_context(
                tc.tile_pool(name="psumA", bufs=2, space="PSUM"))
            psumB = ctx.enter_context(
                tc.tile_pool(name="psumB", bufs=4, space="PSUM"))

            CMAX = max(CHUNKS)
            UCS = 512                   # U2T chunk width (n columns)

            bias_sb = const.tile([P, NTP], F32)
            u2_sb = const.tile([P, RT, OUT], F16)
            u1ts = []                   # U1 chunk tiles, shared by both halves

            def a_load_chunk(h, ck, clen, k0, issue_u1):
                if issue_u1:
                    u1t = u1pool.tile([P, CMAX, R], F16, tag="u1t",
                                      name=f"u1t_{ck}")
                    nc.sync.dma_start(
                        u1t[:, :clen],
                        U1L[k0 * P:(k0 + clen) * P, :]
                        .rearrange("(t p) r -> p t r", p=P),
                    )
                    u1ts.append(u1t)
                xt = xpool.tile([P, CMAX, BH], F16, tag="xt",
                                name=f"xt{h}_{ck}")
                nc.sync.dma_start(
                    xt[:, :clen],
                    xT[k0 * P:(k0 + clen) * P, h * BH:(h + 1) * BH]
                    .rearrange("(t p) b -> p t b", p=P),
                )
                return xt

            def a_matmuls(psA, xt, ck, clen, k0):
                u1t = u1ts[ck]
                for t in range(clen):
                    k = k0 + t
                    for m in range(RT):
                        nc.tensor.matmul(
                            psA[m][:],
                            u1t[:, t, m * P:(m + 1) * P],
                            xt[:, t, :],
                            start=(k == 0),
                            stop=(k == KT - 1),
                        )

            def new_psA(h):
                return [psumA.tile([P, BH], F32, tag=f"psA{m}",
                                   name=f"psA{h}_{m}") for m in range(RT)]

            def evict_z(h, psA):
                # z eviction: PSUM fp32 -> SBUF fp16, DVE || ACT
                z_sb = [const.tile([P, BH], F16, name=f"z{h}_{m}")
                        for m in range(RT)]
                nc.vector.tensor_copy(z_sb[0][:], psA[0][:])
                nc.scalar.copy(z_sb[1][:], psA[1][:])
                return z_sb

            def stage_b(h, z_sb):
                # y'[n, b] = sum_r U2T[r, n] z[r, b] + bias[n]. Two n-tiles
                # accumulate into one PSUM bank; whole pairs alternate
                # between DVE (one fused 512-wide tensor_tensor with a
                # 3D-broadcast bias) and ACT (two activation ops with
                # per-partition bias) so the engines run concurrently and
                # per-op fixed costs amortize. One store per pair.
                # Store groups of 2 pairs (4 n-tiles, 262KB): the whole
                # group is evicted by ONE engine (groups alternate DVE/ACT)
                # so same-tile writes serialize only within an engine, and
                # the store-DMA descriptor-gen rate (~1 per 625-1200ns)
                # keeps up with the ~1.6us group cadence. The final two
                # pairs of the last half are single-pair groups split
                # across both engines (DVE last - fused, fastest) so the
                # kernel tail is one small eviction + one small store.
                if h == 0:
                    sched = [(2, g % 2) for g in range(8)]
                else:
                    # tail: last two pairs are single-pair groups (ACT then
                    # DVE-fused) so the final eviction + store are small
                    sched = [(2, g % 2) for g in range(7)] + [(1, 1), (1, 0)]
                pg = 0
                for g, (npairs, eng) in enumerate(sched):
                    if eng == 2:
                        nt0 = 2 * pg
                        ps = psumB.tile([P, 2, BH], F32, tag="psB")
                        for j in (0, 1):
                            for kr in range(RT):
                                nc.tensor.matmul(
                                    ps[:, j],
                                    u2_sb[:, kr,
                                          (nt0 + j) * P:(nt0 + j + 1) * P],
                                    z_sb[kr][:],
                                    start=(kr == 0),
                                    stop=(kr == RT - 1),
                                )
                        ya = ypool.tile([P, 1, BH], F16, tag="y0",
                                        name=f"yf0_{h}")
                        yb = ypool.tile([P, 1, BH], F16, tag="y1",
                                        name=f"yf1_{h}")
                        nc.vector.tensor_tensor(
                            ya[:, 0], ps[:, 0],
                            bias_sb[:, nt0:nt0 + 1].to_broadcast((P, BH)),
                            mybir.AluOpType.add,
                        )
                        nc.scalar.activation(
                            yb[:, 0], ps[:, 1],
                            mybir.ActivationFunctionType.Identity,
                            bias=bias_sb[:, nt0 + 1:nt0 + 2],
                        )
                        for t, y_one in ((0, ya), (1, yb)):
                            nc.sync.dma_start(
                                y[(nt0 + t) * P:(nt0 + t + 1) * P,
                                  h * BH:(h + 1) * BH]
                                .rearrange("(t p) b -> p t b", p=P),
                                y_one[:],
                            )
                        pg += 1
                        continue
                    y_sb = ypool.tile([P, 2 * npairs, BH], F16,
                                      tag=f"y{eng}", name=f"y{h}_{g}")
                    nt_g0 = 2 * pg
                    for pi in range(npairs):
                        nt0 = 2 * pg
                        ps = psumB.tile([P, 2, BH], F32, tag="psB")
                        for j in (0, 1):
                            for kr in range(RT):
                                nc.tensor.matmul(
                                    ps[:, j],
                                    u2_sb[:, kr,
                                          (nt0 + j) * P:(nt0 + j + 1) * P],
                                    z_sb[kr][:],
                                    start=(kr == 0),
                                    stop=(kr == RT - 1),
                                )
                        off = 2 * pi
                        if eng == 0:
                            nc.vector.tensor_tensor(
                                y_sb[:, off:off + 2], ps[:],
                                bias_sb[:, nt0:nt0 + 2]
                                .to_broadcast((P, 2, BH)),
                                mybir.AluOpType.add,
                            )
                        else:
                            for j in (0, 1):
                                nc.scalar.activation(
                                    y_sb[:, off + j], ps[:, j],
                                    mybir.ActivationFunctionType.Identity,
                                    bias=bias_sb[:, nt0 + j:nt0 + j + 1],
                                )
                        pg += 1
                    dst = (y[nt_g0 * P:2 * pg * P, h * BH:(h + 1) * BH]
                           .rearrange("(t p) b -> p t b", p=P))
                    # h1: alternate Pool/SP store queues so descriptor gen
                    # (SWDGE 1.17us, HWDGE 0.63us) keeps pace with the
                    # ~0.85us eviction cadence. h2: SP only.
                    if h == 0 and g % 2 == 0:
                        nc.gpsimd.dma_start(dst, y_sb[:])
                    else:
                        nc.sync.dma_start(dst, y_sb[:])

            # Pipeline order: [x1/U1 -> A1] [x2 -> A2] [U2T -> B1 -> B2].
            # x2 streams while A1 computes, so A2 follows A1 back-to-back on
            # PE; U2T lands during A2, just ahead of B1's consumption; B1/B2
            # evictions+stores then keep the DMA queue fed to the end.
            psA1 = new_psA(0)
            psA2 = new_psA(1)
            k0 = 0
            for ck, clen in enumerate(CHUNKS):
                xt = a_load_chunk(0, ck, clen, k0, issue_u1=True)
                a_matmuls(psA1, xt, ck, clen, k0)
                k0 += clen
            z1 = evict_z(0, psA1)
            nc.sync.dma_start(bias_sb[:], biasP)
            # half 2 x DMAs issue now (SP program order), matmuls right after
            x2_tiles = []
            k0 = 0
            for ck, clen in enumerate(CHUNKS):
                x2_tiles.append((a_load_chunk(1, ck, clen, k0, False),
                                 ck, clen, k0))
                k0 += clen
            for xt, ck, clen, k0 in x2_tiles:
                a_matmuls(psA2, xt, ck, clen, k0)
            z2 = evict_z(1, psA2)
            # U2T loads issue after x2; FIFO transfer order means chunk ci
            # lands just ahead of stage B1 consuming it.
            for ci in range(OUT // UCS):
                sl = slice(ci * UCS, (ci + 1) * UCS)
                nc.sync.dma_start(u2_sb[:, :, sl], U2T_r[:, :, sl])
            stage_b(0, z1)
            stage_b(1, z2)

    nc.compile()
    return nc


# --------------------------------------------------------------------------
# legacy fp32-class kernels (fallback modes)
# --------------------------------------------------------------------------

def _stage_a_matmuls(nc, ps, lhs_tiles, rhs_tiles, k, last_k):
    """Accumulate all operand-split combinations for one k tile into ps."""
    combos = []
    if len(lhs_tiles) == 1:
        combos = [(0, 0)]
    else:  # hi*hi, hi*lo, lo*hi
        combos = [(0, 0), (0, 1), (1, 0)]
    for ci, (li, ri) in enumerate(combos):
        nc.tensor.matmul(
            ps, lhs_tiles[li], rhs_tiles[ri],
            start=(k == 0 and ci == 0),
            stop=(k == last_k and ci == len(combos) - 1),
        )


def build_kernel(mode=None):
    mode = mode or MODE
    if mode == "fp16":
        return build_kernel_fp16()
    nc = bacc.Bacc(
        "TRN2", target_bir_lowering=False, debug=False, enable_asserts=False
    )
    mm_dt = {"fp32": F32, "fp32r": F32R, "bf16x3": BF16, "fp32rc": F32R}[mode]
    nsplit = 2 if mode in ("bf16x3", "fp32rc") else 1
    # fp32rc: all inputs arrive as single fp32 streams (no DMA inflation);
    # hi/lo Dekker splits into f32r happen on-device, with the copy/sub
    # passes balanced across POOL and DVE.
    x_dev_split = mode == "fp32rc"
    x_streams = 2 if (nsplit == 2 and not x_dev_split) else 1
    w_streams = 1 if x_dev_split else nsplit
    x_dt = F32 if x_dev_split else mm_dt
    w_dt = F32 if x_dev_split else mm_dt

    if x_streams == 1:
        xTs = [nc.dram_tensor("xT", [IN, BSH], x_dt, kind="ExternalInput").ap()]
    else:
        xTs = [nc.dram_tensor(f"xT_{sfx}", [IN, BSH], mm_dt,
                              kind="ExternalInput").ap() for sfx in ("hi", "lo")]
    if w_streams == 1:
        U1s = [nc.dram_tensor("U1", [IN, R], w_dt, kind="ExternalInput").ap()]
        U2Ts = [nc.dram_tensor("U2T", [R, OUT], w_dt, kind="ExternalInput").ap()]
    else:
        U1s = [nc.dram_tensor(f"U1_{sfx}", [IN, R], mm_dt,
                              kind="ExternalInput").ap() for sfx in ("hi", "lo")]
        U2Ts = [nc.dram_tensor(f"U2T_{sfx}", [R, OUT], mm_dt,
                               kind="ExternalInput").ap() for sfx in ("hi", "lo")]
    lam = nc.dram_tensor("lam", [R], F32, kind="ExternalInput").ap()
    bias = nc.dram_tensor("bias", [OUT], F32, kind="ExternalInput").ap()
    y = nc.dram_tensor("y", [BSH, OUT], F32, kind="ExternalOutput").ap()

    with tile.TileContext(nc) as tc:
        with ExitStack() as ctx:
            const = ctx.enter_context(tc.tile_pool(name="const", bufs=1))
            raw_bufs = 2 if x_dev_split else 6
            xpool = ctx.enter_context(tc.tile_pool(name="xpool", bufs=raw_bufs))
            wpool = ctx.enter_context(tc.tile_pool(name="wpool", bufs=raw_bufs))
            spool = ctx.enter_context(
                tc.tile_pool(name="spool", bufs=3))
            u2rpool = ctx.enter_context(tc.tile_pool(name="u2rpool", bufs=2))
            zpool = ctx.enter_context(tc.tile_pool(name="zpool", bufs=1))
            ypool = ctx.enter_context(
                tc.tile_pool(name="ypool", bufs=3 if x_dev_split else 4))
            psumA = ctx.enter_context(tc.tile_pool(name="psumA", bufs=1, space="PSUM"))
            psumB = ctx.enter_context(tc.tile_pool(name="psumB", bufs=4, space="PSUM"))

            # --- stage A: z[r, b] = sum_k U1[k, r] xT[k, b]  (K = IN = 4096)
            # DMAs are batched into multi-k-tile chunks: descriptor-gen cost
            # (~625ns/DMA on the shared HWDGE) is per dma_start, so fewer +
            # bigger transfers keep the DMA pipeline at bandwidth. The first
            # chunk is small so PE starts early. Constant loads (u2, bias,
            # lam) are sprinkled in so they fill otherwise-idle DMA time.
            CHUNKS = [1, 3] + [4] * 7  # k-tiles per DMA chunk, sum = KT
            CMAX = max(CHUNKS)
            psA = [psumA.tile([P, BSH], F32, name=f"psA{m}") for m in range(RT)]
            lam_sb = const.tile([P, RT], F32)
            bias_bc = const.tile([P, OUT], F32)
            u2_sb = [const.tile([P, RT, OUT], mm_dt, tag=f"u2{s}", name=f"u2{s}")
                     for s in range(nsplit)]
            U2T_r = [u.rearrange("(kt p) n -> p kt n", p=P) for u in U2Ts]
            # u2 load chunk width. For fp32rc most u2 split work is deferred
            # into stage B (where DVE is otherwise idle), chunk = one n-tile.
            csz = 512 if x_dev_split else 1024
            u2_jobs = [(s, ci) for s in range(w_streams)
                       for ci in range(OUT // csz)]
            n_u2_stage_a = 3 if x_dev_split else len(u2_jobs)

            def load_u2_chunk(s, ci):
                sl = slice(ci * csz, (ci + 1) * csz)
                if not x_dev_split:
                    nc.sync.dma_start(u2_sb[s][:, :, sl], U2T_r[s][:, :, sl])
                    return
                raw = u2rpool.tile([P, RT, csz], F32, tag="u2raw",
                                   name=f"u2raw_{ci}")
                nc.sync.dma_start(raw[:], U2T_r[0][:, :, sl])
                nc.vector.tensor_copy(u2_sb[0][:, :, sl], raw[:])
                nc.vector.tensor_tensor(
                    u2_sb[1][:, :, sl], raw[:], u2_sb[0][:, :, sl],
                    mybir.AluOpType.subtract,
                )

            k0 = 0
            for ck, clen in enumerate(CHUNKS):
                xts, u1ts = [], []
                for s in range(x_streams):
                    xt = xpool.tile([P, CMAX, BSH], x_dt, tag=f"xt{s}",
                                    name=f"xt{s}_{ck}")
                    nc.sync.dma_start(
                        xt[:, :clen],
                        xTs[s][k0 * P:(k0 + clen) * P, :]
                        .rearrange("(t p) b -> p t b", p=P),
                    )
                    xts.append(xt)
                for s in range(w_streams):
                    u1t = wpool.tile([P, CMAX, R], w_dt, tag=f"u1{s}",
                                     name=f"u1{s}_{ck}")
                    nc.sync.dma_start(
                        u1t[:, :clen],
                        U1s[s][k0 * P:(k0 + clen) * P, :]
                        .rearrange("(t p) r -> p t r", p=P),
                    )
                    u1ts.append(u1t)
                if x_dev_split:
                    # Dekker split on device (DVE): hi = rnd_f32r(x),
                    # lo = rnd_f32r(x - hi); chunk-granular ops.
                    xh = spool.tile([P, CMAX, BSH], F32R, tag="xh",
                                    name=f"xh_{ck}")
                    xl = spool.tile([P, CMAX, BSH], F32R, tag="xl",
                                    name=f"xl_{ck}")
                    u1h = u2rpool.tile([P, CMAX, R], F32R, tag="u1h",
                                       name=f"u1h_{ck}")
                    u1l = u2rpool.tile([P, CMAX, R], F32R, tag="u1l",
                                       name=f"u1l_{ck}")
                    nc.vector.tensor_copy(xh[:, :clen], xts[0][:, :clen])
                    nc.vector.tensor_tensor(
                        xl[:, :clen], xts[0][:, :clen], xh[:, :clen],
                        mybir.AluOpType.subtract,
                    )
                    nc.vector.tensor_copy(u1h[:, :clen], u1ts[0][:, :clen])
                    nc.vector.tensor_tensor(
                        u1l[:, :clen], u1ts[0][:, :clen], u1h[:, :clen],
                        mybir.AluOpType.subtract,
                    )
                    xts = [xh, xl]
                    u1ts = [u1h, u1l]
                if ck == 1:
                    nc.sync.dma_start(lam_sb[:], lam.rearrange("(o p) -> p o", p=P))
                    nc.sync.dma_start(bias_bc[0:1, :],
                                      bias.rearrange("(a n) -> a n", a=1))
                if ck == (len(CHUNKS) - 1 if x_dev_split else 2):
                    nc.gpsimd.partition_broadcast(bias_bc[:], bias_bc[0:1, :])
                if ck >= 3 and u2_jobs and (
                        (OUT // csz) * w_streams - len(u2_jobs) < n_u2_stage_a):
                    load_u2_chunk(*u2_jobs.pop(0))
                    if ck == len(CHUNKS) - 1 and not x_dev_split:
                        while u2_jobs:  # flush any leftovers (bf16x3)
                            load_u2_chunk(*u2_jobs.pop(0))
                for t in range(clen):
                    k = k0 + t
                    for m in range(RT):
                        _stage_a_matmuls(
                            nc, psA[m][:],
                            [u[:, t, m * P:(m + 1) * P] for u in u1ts],
                            [x[:, t, :] for x in xts],
                            k, KT - 1,
                        )
                k0 += clen

            # --- z eviction: z = psA * lam  (+ hi/lo split on device for bf16x3)
            z_sb = []
            if nsplit == 1:
                zt = zpool.tile([P, RT, BSH], mm_dt, tag="z")
                for m in range(RT):
                    nc.vector.tensor_tensor(
                        zt[:, m], psA[m][:],
                        lam_sb[:, m:m + 1].to_broadcast((P, BSH)),
                        mybir.AluOpType.mult,
                    )
                z_sb = [zt]
            else:
                lo_dt = F32R if x_dev_split else BF16
                zf = zpool.tile([P, RT, BSH], F32, tag="zf")
                z_hi = zpool.tile([P, RT, BSH], lo_dt, tag="zhi")
                z_lo = zpool.tile([P, RT, BSH], lo_dt, tag="zlo")
                for m in range(RT):
                    nc.vector.tensor_tensor(
                        zf[:, m], psA[m][:],
                        lam_sb[:, m:m + 1].to_broadcast((P, BSH)),
                        mybir.AluOpType.mult,
                    )
                    nc.vector.tensor_copy(z_hi[:, m], zf[:, m])
                    nc.vector.tensor_tensor(
                        z_lo[:, m], zf[:, m], z_hi[:, m],
                        mybir.AluOpType.subtract,
                    )
                z_sb = [z_hi, z_lo]

            # --- stage B: y[b, n] = sum_r z[r, b] U2T[r, n] + bias[n]
            # y goes out in [P, 2048] chunks via gpsimd (SWDGE) so store
            # descriptor-gen doesn't contend with the HWDGE load pipeline.
            combos = [(0, 0)] if nsplit == 1 else [(0, 0), (0, 1), (1, 0)]
            YC = 1024  # columns per output store
            for mb in range(MB):
                y_sb = None
                for n in range(NT):
                    # fp32rc: remaining u2 chunks split just-in-time here —
                    # chunk n+2 is produced while tiles (mb=0, n) compute,
                    # keeping the big u2 split off stage A's critical DVE path.
                    if u2_jobs:
                        load_u2_chunk(*u2_jobs.pop(0))
                    ps = psumB.tile([P, 512], F32, tag="psB")
                    for kr in range(RT):
                        for ci, (li, ri) in enumerate(combos):
                            nc.tensor.matmul(
                                ps[:],
                                z_sb[li][:, kr, mb * P:(mb + 1) * P],
                                u2_sb[ri][:, kr, n * 512:(n + 1) * 512],
                                start=(kr == 0 and ci == 0),
                                stop=(kr == RT - 1 and ci == len(combos) - 1),
                            )
                    # last row-block streams out per 512-col tile so the
                    # kernel tail is one small store, not a 1MB one
                    yc = 512 if (mb == MB - 1 and mm_dt == F32) else YC
                    if n % (yc // 512) == 0:
                        y_sb = ypool.tile([P, yc], F32, tag="y", name=f"y_{mb}_{n}")
                    off = (n % (yc // 512)) * 512
                    nc.vector.tensor_tensor(
                        y_sb[:, off:off + 512], ps[:],
                        bias_bc[:, n * 512:(n + 1) * 512],
                        mybir.AluOpType.add,
                    )
                    if (n + 1) % (yc // 512) == 0:
                        nc.gpsimd.dma_start(
                            y[mb * P:(mb + 1) * P,
                              (n + 1) * 512 - yc:(n + 1) * 512],
                            y_sb[:, :yc],
                        )

    nc.compile()
    return nc


def _round_f32r(a):
    """Round fp32 -> tf32-like (11 mantissa bits), round-half-to-even."""
    bits = a.view(np.uint32)
    rounded = (bits.astype(np.uint64) + 0x7FF + ((bits >> 12) & 1)) & 0xFFFFF000
    return rounded.astype(np.uint32).view(np.float32)


def _split_bf16(a):
    import ml_dtypes
    hi = a.astype(ml_dtypes.bfloat16)
    lo = (a - hi.astype(np.float32)).astype(ml_dtypes.bfloat16)
    return hi, lo


def prep_in_maps(x, U1, U2, lam, bias, mode=None):
    mode = mode or MODE
    x = np.ascontiguousarray(np.asarray(x, dtype=np.float32))
    U1 = np.ascontiguousarray(np.asarray(U1, dtype=np.float32))
    U2T = np.ascontiguousarray(np.asarray(U2, dtype=np.float32).T)
    lam = np.ascontiguousarray(np.asarray(lam, dtype=np.float32))
    bias = np.ascontiguousarray(np.asarray(bias, dtype=np.float32))

    if mode == "fp16":
        U1L16 = np.ascontiguousarray((U1 * lam[None, :]).astype(np.float16))
        U2T16 = np.ascontiguousarray(U2T.astype(np.float16))
        biasP = np.ascontiguousarray(bias.reshape(NTP, P).T)  # [128, 32] f32
        in_maps = []
        for i in range(NCORES):
            xT16 = np.ascontiguousarray(
                x[i * BSH:(i + 1) * BSH, :].T.astype(np.float16))
            in_maps.append({"xT": xT16, "U1L": U1L16, "U2T": U2T16,
                            "biasP": biasP})
        return in_maps

    in_maps = []
    for i in range(NCORES):
        xT_i = np.ascontiguousarray(x[i * BSH:(i + 1) * BSH, :].T)
        if mode == "fp32":
            m = {"xT": xT_i, "U1": U1, "U2T": U2T}
        elif mode == "fp32rc":
            m = {"xT": xT_i, "U1": U1, "U2T": U2T}
        elif mode == "fp32r":
            m = {"xT": _round_f32r(xT_i), "U1": _round_f32r(U1),
                 "U2T": _round_f32r(U2T)}
        else:
            xh, xl = _split_bf16(xT_i)
            u1h, u1l = _split_bf16(U1)
            u2h, u2l = _split_bf16(U2T)
            m = {"xT_hi": xh, "xT_lo": xl, "U1_hi": u1h, "U1_lo": u1l,
                 "U2T_hi": u2h, "U2T_lo": u2l}
        m["lam"] = lam
        m["bias"] = bias
        in_maps.append(m)
    return in_maps


def postprocess(raw, mode=None):
    """raw: concat over cores along axis 0 of each core's output tensor."""
    mode = mode or MODE
    if mode != "fp16":
        return raw  # already [B, OUT] fp32 in batch order
    out = np.empty((B, OUT), np.float32)
    for i in range(NCORES):
        out[i * BSH:(i + 1) * BSH, :] = raw[i * OUT:(i + 1) * OUT, :].T
    return out


_NC_CACHE = {}


def _build_runner(nc):
    """PJRT runner WITHOUT output-buffer donation.

    The stock run_bass_via_pjrt path donates zero-initialized output
    buffers into the bass_exec custom call; on this axon stack that
    donation intermittently corrupted outputs or crashed the exec unit
    (~1 in 5 fresh-process runs for larger-input kernels). With donation
    off (fresh result buffers, 23/23 clean trials), execution is
    deterministic. Our kernel writes every output element, so the zero
    initial contents are irrelevant.
    """
    import jax
    from jax.sharding import Mesh, PartitionSpec, NamedSharding
    from jax.experimental.shard_map import shard_map
    from concourse import bass2jax

    bass2jax.install_neuronx_cc_hook()
    partition_name = nc.partition_id_tensor.name if nc.partition_id_tensor else None
    in_names, out_names, out_avals, zero_outs = [], [], [], []
    for alloc in nc.m.functions[0].allocations:
        if not isinstance(alloc, mybir.MemoryLocationSet):
            continue
        name = alloc.memorylocations[0].name
        if alloc.kind == "ExternalInput":
            if name != partition_name:
                in_names.append(name)
        elif alloc.kind == "ExternalOutput":
            out_names.append(name)
            shape = tuple(alloc.tensor_shape)
            dtype = mybir.dt.np(alloc.dtype)
            out_avals.append(jax.core.ShapedArray(shape, dtype))
            zero_outs.append(np.zeros(shape, dtype))
    all_in_names = list(in_names) + list(out_names)
    if partition_name is not None:
        all_in_names.append(partition_name)

    def _body(*args):
        operands = list(args)
        if partition_name is not None:
            operands.append(bass2jax.partition_id_tensor())
        return tuple(bass2jax._bass_exec_p.bind(
            *operands,
            out_avals=tuple(out_avals),
            in_names=tuple(all_in_names),
            out_names=tuple(out_names),
            lowering_input_output_aliases=(),
            sim_require_finite=True,
            sim_require_nnan=True,
            nc=nc,
        ))

    devices = jax.devices()[:NCORES]
    mesh = Mesh(np.asarray(devices), ("core",))
    nin = len(in_names) + len(zero_outs)
    fn = jax.jit(
        shard_map(_body, mesh=mesh,
                  in_specs=(PartitionSpec("core"),) * nin,
                  out_specs=(PartitionSpec("core"),) * len(out_names),
                  check_rep=False),
        keep_unused=True,
    )
    sharding = NamedSharding(mesh, PartitionSpec("core"))
    dev_zeros = [
        jax.device_put(
            np.zeros((NCORES * z.shape[0], *z.shape[1:]), z.dtype), sharding)
        for z in zero_outs
    ]

    def run(in_maps):
        concat_in = [
            jax.device_put(
                np.concatenate([np.asarray(in_maps[c][nm])
                                for c in range(NCORES)], axis=0), sharding)
            for nm in in_names
        ]
        outs = fn(*concat_in, *dev_zeros)
        return np.asarray(outs[0])  # concat over cores along axis 0

    return run


_BASS_CACHE = {}


def _run_once(mode, x, U1, U2, lam, bias, _trace, _tmpdir):
    if mode not in _NC_CACHE:
        nc = build_kernel(mode)
        _BASS_CACHE[mode] = nc
        _NC_CACHE[mode] = _build_runner(nc)
    in_maps = prep_in_maps(x, U1, U2, lam, bias, mode)
    return postprocess(_NC_CACHE[mode](in_maps), mode)


def kernel(x, U1, U2, lam, bias, _trace=False, _tmpdir=None, _mode=None):
    # Device execution through the axon tunnel can very occasionally fail
    # transiently (NRT_EXEC_UNIT_UNRECOVERABLE); retry the same mode, then
    # fall back to the legacy fp32-class kernels before giving up.
    mode = _mode or MODE
    attempts = [mode, mode, "fp32rc", "fp32"]
    last_err = None
    for i, m in enumerate(attempts):
        try:
            return _run_once(m, x, U1, U2, lam, bias, _trace, _tmpdir)
        except Exception as e:  # noqa: BLE001 - deliberate retry barrier
            last_err = e
            import time as _time
            _time.sleep(2.0 * (i + 1))
    raise last_err
